# revision 1
# baseline (speedup 1.0000x reference)
"""Multi-head self-attention (B=2, N=2048, D=1024, H=16, Dh=64) on 8 TRN2 NeuronCores.

Sharding: core c handles batch b = c // 4 and head group g = c % 4 (heads 4g..4g+3).
Tensor-parallel on heads for qkv/out_proj; data-parallel on batch. Each core
produces a partial [D, N] output (transposed); host sums the 4 head-group
partials per batch, transposes, and adds b_out.

All matmuls run in float32r (TF32-like PE mode, 1 cyc/row) with fp32 PSUM
accumulation; measured end-to-end relative error ~1e-4.
"""
import sys
import numpy as np

for _p in ("/opt/trn_rl_repo", "/root/.axon_site/_ro/trn_rl_repo"):
    if _p not in sys.path:
        sys.path.append(_p)

import concourse.bass as bass
import concourse.bacc as bacc
import concourse.tile as tile
from concourse import mybir
from concourse.bass_utils import run_bass_kernel_spmd

F32 = mybir.dt.float32
F32R = mybir.dt.float32r
EXP = mybir.ActivationFunctionType.Exp

B, S, D = 2, 2048, 1024
H, DH = 16, 64
HL = 4            # heads per core (local)
CQK = 512         # q+k channels per core (2*HL*DH)
CV = 256          # v channels per core (HL*DH)
ND = D // 128     # 8 d-tiles
NKT = S // 128    # 16 key tiles
NQC = S // 512    # 4 query chunks of 512


def build_kernel() -> "bass.Bass":
    nc = bacc.Bacc(None, target_bir_lowering=False, debug=False)

    xT = nc.dram_tensor("xT", [D, S], F32R, kind="ExternalInput")
    wqk = nc.dram_tensor("wqk", [D, CQK], F32R, kind="ExternalInput")
    bqk = nc.dram_tensor("bqk", [128, CQK // 128], F32, kind="ExternalInput")
    wv = nc.dram_tensor("wv", [D, CV], F32R, kind="ExternalInput")
    bvb = nc.dram_tensor("bvb", [128, CV], F32, kind="ExternalInput")
    wout = nc.dram_tensor("wout", [CV, D], F32R, kind="ExternalInput")
    outT = nc.dram_tensor("outT", [D, S], F32, kind="ExternalOutput")

    xT_r = xT.rearrange("(t p) s -> t p s", p=128)        # [8, 128, 2048]
    wqk_r = wqk.rearrange("(t p) c -> t p c", p=128)      # [8, 128, 512]
    wv_r = wv.rearrange("(t p) c -> t p c", p=128)        # [8, 128, 256]
    wout_r = wout.rearrange("(t p) n -> t p n", p=128)    # [2, 128, 1024]
    outT_r = outT.rearrange("(t p) s -> t p s", p=128)    # [8, 128, 2048]

    with tile.TileContext(nc) as tc:
        with tc.tile_pool(name="persist", bufs=1) as persist:
            qkt_s = persist.tile([128, 4, S], F32R)          # [c-tile(q0 q1 k0 k1), seq]
            v_s = persist.tile([128, NKT, HL, DH + 1], F32R)  # per key-tile V + ones col
            at_s = persist.tile([128, 2, S], F32R)            # normalized attn out^T
            wout_s = persist.tile([128, 2, D], F32R)
            bqk_s = persist.tile([128, CQK // 128], F32)
            bvb_s = persist.tile([128, CV], F32)

            for t in range(2):
                nc.sync.dma_start(out=wout_s[:, t, :], in_=wout_r[t])
            nc.sync.dma_start(out=bqk_s[:], in_=bqk[:])
            nc.sync.dma_start(out=bvb_s[:], in_=bvb[:])
            nc.vector.memset(v_s[:, :, :, DH:DH + 1].bitcast(F32), 1.0)

            # ---------------- Phase A: projections ----------------
            pha_ctx = [tc.tile_pool(name="phA", bufs=1),
                       tc.tile_pool(name="phA_ps", bufs=2, space="PSUM")]
            pha, pps = [c.__enter__() for c in pha_ctx]
            pps2 = pps    # A2 shares A1's psum tags (sequential use, 2x-buffered)
            xt_s = pha.tile([128, ND, S], F32R)
            wqk_s = pha.tile([128, ND, CQK], F32R)
            wv_s = pha.tile([128, ND, CV], F32R)
            # first d-tile split in halves so A1's first matmuls gate on
            # ~0.6MB instead of 1.25MB; remaining tiles load whole
            nc.sync.dma_start(out=wqk_s[:, 0, 0:256], in_=wqk_r[0][:, 0:256])
            nc.sync.dma_start(out=xt_s[:, 0, 0:1024], in_=xT_r[0][:, 0:1024])
            nc.sync.dma_start(out=wqk_s[:, 0, 256:512], in_=wqk_r[0][:, 256:512])
            nc.sync.dma_start(out=xt_s[:, 0, 1024:2048], in_=xT_r[0][:, 1024:2048])
            nc.sync.dma_start(out=wv_s[:, 0, :], in_=wv_r[0])
            for t in range(1, ND):
                nc.sync.dma_start(out=wqk_s[:, t, :], in_=wqk_r[t])
                nc.sync.dma_start(out=xt_s[:, t, :], in_=xT_r[t])
                nc.sync.dma_start(out=wv_s[:, t, :], in_=wv_r[t])

            # A1: q,k projection -> qkt_s (transposed layout, bias added, q pre-scaled)
            for m in (0, 2, 1, 3):
                ps = [pps.tile([128, 512], F32, tag=f"qk{n}", name=f"psqk{n}")
                      for n in range(NQC)]
                for d in range(ND):
                    lhsT = wqk_s[:, d, m * 128:(m + 1) * 128]
                    for n in range(NQC):
                        nc.tensor.matmul(ps[n][:], lhsT,
                                         xt_s[:, d, n * 512:(n + 1) * 512],
                                         start=(d == 0), stop=(d == ND - 1))
                for n in range(NQC):
                    nc.vector.tensor_scalar_add(
                        qkt_s[:, m, n * 512:(n + 1) * 512], ps[n][:],
                        bqk_s[:, m:m + 1])

            # A2: v projection, 4 interleaved PSUM chains
            for st0 in range(0, NKT, 4):
                psvs = [pps2.tile([128, CV], F32, tag=f"qk{j}", name=f"psv{j}")
                        for j in range(4)]
                for d in range(ND):
                    for j in range(4):
                        st = st0 + j
                        nc.tensor.matmul(psvs[j][:],
                                         xt_s[:, d, st * 128:(st + 1) * 128],
                                         wv_s[:, d, :],
                                         start=(d == 0), stop=(d == ND - 1))
                for j in range(4):
                    st = st0 + j
                    nc.vector.tensor_tensor(
                        out=v_s[:, st, :, 0:DH],
                        in0=psvs[j][:].rearrange("p (h c) -> p h c", h=HL),
                        in1=bvb_s[:].rearrange("p (h c) -> p h c", h=HL),
                        op=mybir.AluOpType.add)

            for c in reversed(pha_ctx):
                c.__exit__(None, None, None)

            # ---------------- Phase B ----------------
            # qb=512 blocks; one [128,1024] S^T psum holds BOTH heads of the
            # pair side by side, so exp still runs 1024-wide on ACT while the
            # psum is triple-buffered -> no pipeline-fill bubble (a PE dip at
            # block boundaries re-throttles the HAM clock and halves B).
            stage_ctx = tc.tile_pool(name="stage", bufs=6)
            stage = stage_ctx.__enter__()
            bc_ctx = [tc.tile_pool(name="ptp", bufs=6),
                      tc.tile_pool(name="smallB", bufs=3),
                      tc.tile_pool(name="phB_s", bufs=3, space="PSUM"),
                      tc.tile_pool(name="phB_av", bufs=1, space="PSUM")]
            ptp, small, psb, psav = [c.__enter__() for c in bc_ctx]
            for p in range(2):      # head pairs (2p, 2p+1)
                qt = qkt_s[:, p, :]
                kt = qkt_s[:, 2 + p, :]
                for qb in range(NQC):   # query blocks of 512, C fused after p1
                    q0 = qb * 512
                    qs = slice(q0, q0 + 512)
                    pA = psav.tile([DH + 1, 512], F32, tag="pA", name="pA")
                    pB = psav.tile([DH + 1, 512], F32, tag="pB", name="pB")
                    for t in range(NKT):
                        sAB = psb.tile([128, 1024], F32, tag="sAB", name="sAB")
                        nc.tensor.matmul(sAB[:, 0:512],
                                         kt[0:64, t * 128:(t + 1) * 128],
                                         qt[0:64, qs], start=True, stop=True,
                                         tile_position=(0, 0))
                        nc.tensor.matmul(sAB[:, 512:1024],
                                         kt[64:128, t * 128:(t + 1) * 128],
                                         qt[64:128, qs], start=True, stop=True,
                                         tile_position=(64, 0))
                        pt = ptp.tile([128, 1024], F32R, tag="pt", name="pt")
                        nc.scalar.activation(pt[:], sAB[:], EXP)
                        nc.tensor.matmul(pA[:], v_s[:, t, 2 * p, :],
                                         pt[:, 0:512],
                                         start=(t == 0), stop=(t == NKT - 1))
                        nc.tensor.matmul(pB[:], v_s[:, t, 2 * p + 1, :],
                                         pt[:, 512:1024],
                                         start=(t == 0), stop=(t == NKT - 1))
                    # normalize by softmax denominator (row DH of each psum)
                    for loc, pX in ((0, pA), (1, pB)):
                        hh = 2 * p + loc
                        raw = small.tile([DH + 1, 512], F32, tag="raw", name="raw")
                        nc.vector.tensor_copy(out=raw[:], in_=pX[:])
                        dn = small.tile([64, 8], F32, tag="dn", name="dn")
                        nc.sync.dma_start(out=dn[:], in_=raw[DH:DH + 1, :])
                        rr = small.tile([64, 8], F32, tag="rr", name="rr")
                        nc.vector.reciprocal(rr[:], dn[:])
                        r = small.tile([1, 512], F32, tag="r", name="r")
                        nc.sync.dma_start(out=r[:], in_=rr[:])
                        rb = small.tile([64, 512], F32, tag="rb", name="rb")
                        nc.gpsimd.partition_broadcast(rb[:], r[:])
                        if loc == 0:
                            nc.vector.tensor_tensor(
                                out=at_s[0:64, p, qs],
                                in0=raw[0:DH, :], in1=rb[:],
                                op=mybir.AluOpType.mult)
                        else:
                            # DVE lanes cannot shift partitions; bounce via DMA
                            tmp = small.tile([64, 512], F32R, tag="tmp", name="tmp")
                            nc.vector.tensor_tensor(
                                out=tmp[:], in0=raw[0:DH, :], in1=rb[:],
                                op=mybir.AluOpType.mult)
                            nc.sync.dma_start(
                                out=at_s[64:128, p, qs], in_=tmp[:])

            for c in reversed(bc_ctx):
                c.__exit__(None, None, None)

            # ---------------- Phase C: out^T = wout^T @ at ----------------
            c_ctx = [tc.tile_pool(name="phC_ps", bufs=1, space="PSUM")]
            psc = c_ctx[0].__enter__()
            for qc in range(NQC):
                qg = slice(qc * 512, (qc + 1) * 512)
                for nt0 in range(0, ND, 2):
                    pos = [psc.tile([128, 512], F32, tag=f"o{(nt0 // 2) % 2}_{j}",
                                    name=f"po{j}") for j in range(2)]
                    for ct in range(2):
                        for j in range(2):
                            nt = nt0 + j
                            nc.tensor.matmul(pos[j][:],
                                             wout_s[:, ct, nt * 128:(nt + 1) * 128],
                                             at_s[:, ct, qg],
                                             start=(ct == 0), stop=(ct == 1))
                    for j in range(2):
                        nt = nt0 + j
                        o = stage.tile([128, 512], F32, tag="o", name="o")
                        nc.vector.tensor_copy(out=o[:], in_=pos[j][:])
                        nc.sync.dma_start(out=outT_r[nt][:, qg], in_=o[:])
            c_ctx[0].__exit__(None, None, None)
            stage_ctx.__exit__(None, None, None)
    nc.compile()
    return nc


def shard_inputs(x, W_qkv, b_qkv, W_out, b_out=None):
    """Build the 8 per-core input maps. Core c: batch c//4, head group c%4."""
    in_maps = []
    scale = 1.0 / np.sqrt(np.float32(DH))
    for c in range(8):
        b, g = divmod(c, 4)
        cs = slice(g * 256, g * 256 + 256)
        xTc = np.ascontiguousarray(x[b].T)                       # [D, S]
        wq = W_qkv[:, 0:D][:, cs] * scale                        # [D, 256]
        wk = W_qkv[:, D:2 * D][:, cs]
        wqk = np.ascontiguousarray(np.concatenate([wq, wk], axis=1))  # [D, 512]
        bq = b_qkv[0:D][cs] * scale
        bk = b_qkv[D:2 * D][cs]
        bqk = np.concatenate([bq, bk]).reshape(CQK // 128, 128).T     # [128, 4]
        bqk = np.ascontiguousarray(bqk)
        wv = np.ascontiguousarray(W_qkv[:, 2 * D:3 * D][:, cs])       # [D, 256]
        bvb = np.ascontiguousarray(
            np.broadcast_to(b_qkv[2 * D:3 * D][cs], (128, CV)))       # [128, 256]
        woutc = np.ascontiguousarray(W_out[cs, :])                    # [256, D]
        in_maps.append({
            "xT": xTc.astype(np.float32),
            "wqk": wqk.astype(np.float32),
            "bqk": bqk.astype(np.float32),
            "wv": wv.astype(np.float32),
            "bvb": bvb.astype(np.float32),
            "wout": woutc.astype(np.float32),
        })
    return in_maps


_NC_CACHE = []


def _get_nc():
    if not _NC_CACHE:
        _NC_CACHE.append(build_kernel())
    return _NC_CACHE[0]


def run_sharded(in_maps, **kwargs):
    nc = _get_nc()
    return run_bass_kernel_spmd(nc, in_maps, core_ids=list(range(8)), **kwargs)


def gather_output(results, b_out):
    out = np.empty((B, S, D), dtype=np.float32)
    for b in range(B):
        acc = results[4 * b]["outT"].astype(np.float32).copy()
        for g in range(1, 4):
            acc += results[4 * b + g]["outT"]
        out[b] = acc.T + b_out[None, :]
    return out


def kernel(x, W_qkv, b_qkv, W_out, b_out):
    x = np.asarray(x, dtype=np.float32)
    W_qkv = np.asarray(W_qkv, dtype=np.float32)
    b_qkv = np.asarray(b_qkv, dtype=np.float32)
    W_out = np.asarray(W_out, dtype=np.float32)
    b_out = np.asarray(b_out, dtype=np.float32)
    in_maps = shard_inputs(x=x, W_qkv=W_qkv, b_qkv=b_qkv, W_out=W_out, b_out=b_out)
    res = run_sharded(in_maps)
    return gather_output(res.results, b_out)



# revision 2
# speedup vs baseline: 1.1649x; 1.1649x over previous
"""Multi-head self-attention (B=2, N=2048, D=1024, H=16, Dh=64) on 8 TRN2 NeuronCores.

Sharding: core c handles batch b = c // 4 and head group g = c % 4 (heads 4g..4g+3).
Tensor-parallel on heads for qkv/out_proj; data-parallel on batch. Each core
produces a partial [D, N] output (transposed, bf16); host sums the 4 head-group
partials per batch, transposes, and adds b_out.

Fully-fused single-region schedule: the softmax exp on the scalar (ACT) engine
is the pacing resource (~131us of exp at 1.2 GHz, dtype-independent), so the
qkv-projection, v-projection and out-projection matmul chains are emitted as
fillers between attention iterations. That keeps the PE warm (no HAM
re-throttle) and hides phases A/C entirely behind the ACT-bound attention loop.
All SBUF tensors are bf16 (PE rate is identical to f32r, DMA bytes halve);
PSUM accumulation stays fp32.
"""
import sys
import numpy as np
import ml_dtypes

for _p in ("/opt/trn_rl_repo", "/root/.axon_site/_ro/trn_rl_repo"):
    if _p not in sys.path:
        sys.path.append(_p)

import concourse.bass as bass
import concourse.bacc as bacc
import concourse.tile as tile
from concourse import mybir
from concourse.bass_utils import run_bass_kernel_spmd

F32 = mybir.dt.float32
BF16 = mybir.dt.bfloat16
EXP = mybir.ActivationFunctionType.Exp
MULT = mybir.AluOpType.mult
ADD = mybir.AluOpType.add

B, S, D = 2, 2048, 1024
H, DH = 16, 64
HL = 4            # heads per core (local)
CQK = 512         # k+q channels per core (2*HL*DH); dram col order [k0 k1 q0 q1]
CV = 256          # v channels per core (HL*DH)
ND = D // 128     # 8 d-tiles
NKT = S // 128    # 16 key tiles
NQC = S // 512    # 4 query blocks of 512


def build_kernel() -> "bass.Bass":
    nc = bacc.Bacc(None, target_bir_lowering=False, debug=False)

    xT = nc.dram_tensor("xT", [D, S], BF16, kind="ExternalInput")
    wqk = nc.dram_tensor("wqk", [D, CQK], BF16, kind="ExternalInput")
    bqk = nc.dram_tensor("bqk", [128, CQK // 128], F32, kind="ExternalInput")
    wv = nc.dram_tensor("wv", [D, CV], BF16, kind="ExternalInput")
    bvb = nc.dram_tensor("bvb", [128, CV], F32, kind="ExternalInput")
    wout = nc.dram_tensor("wout", [CV, D], BF16, kind="ExternalInput")
    outT = nc.dram_tensor("outT", [D, S], BF16, kind="ExternalOutput")

    xT_r = xT.rearrange("(t p) s -> t p s", p=128)        # [8, 128, 2048]
    wqk_r = wqk.rearrange("(t p) c -> t p c", p=128)      # [8, 128, 512]
    wv_r = wv.rearrange("(t p) c -> t p c", p=128)        # [8, 128, 256]
    wout_r = wout.rearrange("(t p) n -> t p n", p=128)    # [2, 128, 1024]
    outT_r = outT.rearrange("(t p) s -> t p s", p=128)    # [8, 128, 2048]

    with tile.TileContext(nc) as tc:
        ctxs = [
            tc.tile_pool(name="persist", bufs=1),
            tc.tile_pool(name="ptp", bufs=8),
            tc.tile_pool(name="small", bufs=3),
            tc.tile_pool(name="stage", bufs=4),
            tc.tile_pool(name="psB", bufs=2, space="PSUM"),
            tc.tile_pool(name="psAV", bufs=1, space="PSUM"),
            tc.tile_pool(name="psF", bufs=2, space="PSUM"),
        ]
        persist, ptp, small, stage, psb, psav, psf = [c.__enter__() for c in ctxs]

        xt_s = persist.tile([128, ND, S], BF16)
        wqk_s = persist.tile([128, ND, CQK], BF16)
        wv_s = persist.tile([128, ND, CV], BF16)
        wout_s = persist.tile([128, 2, D], BF16)
        qkt_s = persist.tile([128, 4, S], BF16)           # m: k0 k1 q0 q1
        v_s = persist.tile([128, NKT, HL, DH + 1], BF16)  # per key-tile V + ones col
        at_s = persist.tile([128, 2, S], BF16)            # normalized attn out^T
        bqk_s = persist.tile([128, CQK // 128], F32)
        bvb_s = persist.tile([128, CV], F32)

        # ---------------- input DMAs, earliest-needed first ----------------
        nc.sync.dma_start(out=bqk_s[:], in_=bqk[:])
        nc.sync.dma_start(out=bvb_s[:], in_=bvb[:])
        nc.vector.memset(v_s[:, :, :, DH:DH + 1], 1.0)
        for d in range(ND):
            nc.sync.dma_start(out=wqk_s[:, d, :], in_=wqk_r[d])
            if d == 0:
                nc.sync.dma_start(out=xt_s[:, 0, 0:1024], in_=xT_r[0][:, 0:1024])
                nc.sync.dma_start(out=xt_s[:, 0, 1024:2048], in_=xT_r[0][:, 1024:2048])
            else:
                nc.sync.dma_start(out=xt_s[:, d, :], in_=xT_r[d])
        for d in range(ND):
            nc.sync.dma_start(out=wv_s[:, d, :], in_=wv_r[d])
        for t2 in range(2):
            nc.sync.dma_start(out=wout_s[:, t2, :], in_=wout_r[t2])

        # ---------------- chain builders (each is one PE filler unit) ------
        def a1_chain(m, n):
            # qkt_s[:, m, n*512:(n+1)*512] = wqk_m^T @ x_chunk + bias
            ps = psf.tile([128, 512], F32, tag="fill", name=f"a1_{m}_{n}")
            for d in range(ND):
                nc.tensor.matmul(ps[:], wqk_s[:, d, m * 128:(m + 1) * 128],
                                 xt_s[:, d, n * 512:(n + 1) * 512],
                                 start=(d == 0), stop=(d == ND - 1))
            nc.vector.tensor_scalar_add(
                qkt_s[:, m, n * 512:(n + 1) * 512], ps[:], bqk_s[:, m:m + 1])

        def a2_chain(st):
            # v_s[:, st] = (x_tile^T @ wv) + bias   (keys on partitions)
            ps = psf.tile([128, CV], F32, tag="fill", name=f"a2_{st}")
            for d in range(ND):
                nc.tensor.matmul(ps[:], xt_s[:, d, st * 128:(st + 1) * 128],
                                 wv_s[:, d, :],
                                 start=(d == 0), stop=(d == ND - 1))
            nc.vector.tensor_tensor(
                out=v_s[:, st, :, 0:DH],
                in0=ps[:].rearrange("p (h c) -> p h c", h=HL),
                in1=bvb_s[:].rearrange("p (h c) -> p h c", h=HL),
                op=ADD)

        def c_chain(qc, nt):
            # outT[nt, qc-block] = wout^T @ at  (contract local 256 channels)
            qg = slice(qc * 512, (qc + 1) * 512)
            ps = psf.tile([128, 512], F32, tag="fill", name=f"c_{qc}_{nt}")
            for ct in range(2):
                nc.tensor.matmul(ps[:], wout_s[:, ct, nt * 128:(nt + 1) * 128],
                                 at_s[:, ct, qg],
                                 start=(ct == 0), stop=(ct == 1))
            o = stage.tile([128, 512], BF16, tag="o", name="o")
            nc.vector.tensor_copy(out=o[:], in_=ps[:])
            nc.sync.dma_start(out=outT_r[nt][:, qg], in_=o[:])

        # ---------------- attention block with interleaved fillers ---------
        def b_block(qb, p, fillers):
            kt = qkt_s[:, p, :]
            qt = qkt_s[:, 2 + p, :]
            q0 = qb * 512
            qs = slice(q0, q0 + 512)
            pA = psav.tile([DH + 1, 512], F32, tag="pA", name="pA")
            pB = psav.tile([DH + 1, 512], F32, tag="pB", name="pB")
            nf = len(fillers)
            fi = 0
            for t in range(NKT):
                sAB = psb.tile([128, 1024], F32, tag="sAB", name="sAB")
                nc.tensor.matmul(sAB[:, 0:512],
                                 kt[0:64, t * 128:(t + 1) * 128],
                                 qt[0:64, qs], start=True, stop=True,
                                 tile_position=(0, 0))
                nc.tensor.matmul(sAB[:, 512:1024],
                                 kt[64:128, t * 128:(t + 1) * 128],
                                 qt[64:128, qs], start=True, stop=True,
                                 tile_position=(64, 0))
                pt = ptp.tile([128, 1024], BF16, tag="pt", name="pt")
                nc.scalar.activation(pt[:], sAB[:], EXP)
                nc.tensor.matmul(pA[:], v_s[:, t, 2 * p, :],
                                 pt[:, 0:512],
                                 start=(t == 0), stop=(t == NKT - 1))
                nc.tensor.matmul(pB[:], v_s[:, t, 2 * p + 1, :],
                                 pt[:, 512:1024],
                                 start=(t == 0), stop=(t == NKT - 1))
                # spread fillers evenly over the 16 iterations
                want = (t + 1) * nf // NKT
                while fi < want:
                    fillers[fi]()
                    fi += 1
            # normalize by softmax denominator (ones-row of each psum)
            for loc, pX in ((0, pA), (1, pB)):
                raw = small.tile([DH + 1, 512], F32, tag="raw", name="raw")
                nc.vector.tensor_copy(out=raw[:], in_=pX[:])
                dn = small.tile([64, 8], F32, tag="dn", name="dn")
                nc.sync.dma_start(out=dn[:], in_=raw[DH:DH + 1, :])
                rr = small.tile([64, 8], F32, tag="rr", name="rr")
                nc.vector.reciprocal(rr[:], dn[:])
                r = small.tile([1, 512], F32, tag="r", name="r")
                nc.sync.dma_start(out=r[:], in_=rr[:])
                rb = small.tile([64, 512], F32, tag="rb", name="rb")
                nc.gpsimd.partition_broadcast(rb[:], r[:])
                if loc == 0:
                    nc.vector.tensor_tensor(
                        out=at_s[0:64, p, qs],
                        in0=raw[0:DH, :], in1=rb[:], op=MULT)
                else:
                    # DVE lanes cannot shift partitions; bounce via DMA
                    tmp = small.tile([64, 512], BF16, tag="tmp", name="tmp")
                    nc.vector.tensor_tensor(
                        out=tmp[:], in0=raw[0:DH, :], in1=rb[:], op=MULT)
                    nc.sync.dma_start(out=at_s[64:128, p, qs], in_=tmp[:])

        # ---------------- prologue: k-p0 (all seq) + q-p0 (qb0) + first v --
        a1_chain(0, 0)
        a1_chain(2, 0)
        a1_chain(0, 1)
        a1_chain(0, 2)
        a1_chain(0, 3)
        for st in range(6):
            a2_chain(st)

        # ---------------- fused main loop ----------------------------------
        def F(fn, *a):
            return lambda: fn(*a)

        plan = {
            (0, 0): [F(a2_chain, st) for st in range(6, 16)]
                    + [F(a1_chain, 1, 0), F(a1_chain, 3, 0)],
            (0, 1): [F(a1_chain, 1, 1), F(a1_chain, 1, 2), F(a1_chain, 1, 3),
                     F(a1_chain, 2, 1), F(a1_chain, 3, 1)],
            (1, 0): [F(c_chain, 0, nt) for nt in range(ND)]
                    + [F(a1_chain, 2, 2), F(a1_chain, 3, 2)],
            (1, 1): [F(a1_chain, 2, 3), F(a1_chain, 3, 3)],
            (2, 0): [F(c_chain, 1, nt) for nt in range(ND)],
            (2, 1): [],
            (3, 0): [F(c_chain, 2, nt) for nt in range(ND)],
            (3, 1): [],
        }
        for qb in range(NQC):
            for p in range(2):
                b_block(qb, p, plan[(qb, p)])
        # epilogue: last out-projection block
        for nt in range(ND):
            c_chain(3, nt)

        for c in reversed(ctxs):
            c.__exit__(None, None, None)
    nc.compile()
    return nc


def shard_inputs(x, W_qkv, b_qkv, W_out, b_out=None):
    """Build the 8 per-core input maps. Core c: batch c//4, head group c%4."""
    in_maps = []
    scale = 1.0 / np.sqrt(np.float32(DH))
    bf16 = ml_dtypes.bfloat16
    for c in range(8):
        b, g = divmod(c, 4)
        cs = slice(g * 256, g * 256 + 256)
        xTc = np.ascontiguousarray(x[b].T)                       # [D, S]
        wq = W_qkv[:, 0:D][:, cs] * scale                        # [D, 256]
        wk = W_qkv[:, D:2 * D][:, cs]
        wqkc = np.ascontiguousarray(np.concatenate([wk, wq], axis=1))  # [D, 512] k first
        bq = b_qkv[0:D][cs] * scale
        bk = b_qkv[D:2 * D][cs]
        bqkc = np.concatenate([bk, bq]).reshape(CQK // 128, 128).T     # [128, 4]
        bqkc = np.ascontiguousarray(bqkc)
        wvc = np.ascontiguousarray(W_qkv[:, 2 * D:3 * D][:, cs])       # [D, 256]
        bvbc = np.ascontiguousarray(
            np.broadcast_to(b_qkv[2 * D:3 * D][cs], (128, CV)))        # [128, 256]
        woutc = np.ascontiguousarray(W_out[cs, :])                     # [256, D]
        in_maps.append({
            "xT": xTc.astype(bf16),
            "wqk": wqkc.astype(bf16),
            "bqk": bqkc.astype(np.float32),
            "wv": wvc.astype(bf16),
            "bvb": bvbc.astype(np.float32),
            "wout": woutc.astype(bf16),
        })
    return in_maps


_NC_CACHE = []


def _get_nc():
    if not _NC_CACHE:
        _NC_CACHE.append(build_kernel())
    return _NC_CACHE[0]


def run_sharded(in_maps, **kwargs):
    nc = _get_nc()
    return run_bass_kernel_spmd(nc, in_maps, core_ids=list(range(8)), **kwargs)


def gather_output(results, b_out):
    out = np.empty((B, S, D), dtype=np.float32)
    for b in range(B):
        acc = results[4 * b]["outT"].astype(np.float32)
        for g in range(1, 4):
            acc = acc + results[4 * b + g]["outT"].astype(np.float32)
        out[b] = acc.T + b_out[None, :]
    return out


def kernel(x, W_qkv, b_qkv, W_out, b_out):
    x = np.asarray(x, dtype=np.float32)
    W_qkv = np.asarray(W_qkv, dtype=np.float32)
    b_qkv = np.asarray(b_qkv, dtype=np.float32)
    W_out = np.asarray(W_out, dtype=np.float32)
    b_out = np.asarray(b_out, dtype=np.float32)
    in_maps = shard_inputs(x=x, W_qkv=W_qkv, b_qkv=b_qkv, W_out=W_out, b_out=b_out)
    res = run_sharded(in_maps)
    return gather_output(res.results, b_out)
